# revision 26
# baseline (speedup 1.0000x reference)
"""MoE (top-2 of 8 experts) Trainium2 Bass kernel — routed compute.

Strategy: token-parallel across 8 NeuronCores (1024 tokens each, no
collectives), but unlike a dense all-expert kernel, each core computes
only the top-2 experts per token (1/4 of the dense FLOPs):

  1. Gating on-chip (bf16 matmul, exact-enough top-2 via DVE max8 /
     match_replace as in the dense baseline).
  2. Routing tables on-chip: per-expert token lists are built with
     gpsimd `sparse_gather` compaction of packed (token_id + w/2)
     values; every expert gets a fixed 384-slot segment (3 x 128-slot
     chunks, ~9σ above the Binomial(1024, 1/4) mean of 256, pads -> -1).
  3. `dma_gather(transpose=True)` pulls the selected token rows from
     HBM directly into the transposed [dpart, dchunk, slot] layout the
     PE needs -- no on-chip transpose.
  4. 24 slot-chunks x 2 O-halves x 8 K-chunks of bf16 matmuls (PSUM
     accumulated), scaled by the per-slot gate weight on DVE.
  5. Bias term sum_e w[n,e]*be[e] is a tiny [8]x[8,O] PE matmul per
     token chunk, written directly to out (init for the scatter).
  6. `dma_scatter_add` accumulates the scaled expert rows into out.
"""

import sys

if "/opt/trn_rl_repo" not in sys.path:
    sys.path.insert(0, "/opt/trn_rl_repo")

import numpy as np

import concourse.bass as bass
import concourse.mybir as mybir
from concourse import bacc
from concourse.bass import ds, ts
from concourse.bass_utils import run_bass_kernel_spmd
from concourse.library_config import mlp as mlp_lib, sparse_gather as sg_lib
from concourse.masks import make_identity
from concourse.tile import TileContext

B, S, D, O, E = 4, 2048, 1024, 1024, 8
N = B * S            # 8192 tokens total
NCORES = 8
NT = N // NCORES     # 1024 tokens per core
P = 128
KCH = D // P         # 8 contraction chunks
TCH = NT // P        # 8 token chunks per core
OH = O // 512        # 2 output halves (512 = fp32 PSUM bank)

SLOTS_E = 384        # slots per expert (3 x 128, pads clamped to token 0)
CPE = SLOTS_E // P   # 3 chunks per expert
NSLOT = E * SLOTS_E  # 3072
NCHUNK = NSLOT // P  # 24
EG = 2               # experts per gather/scatter group
NGRP = E // EG       # 4
GSLOT = EG * SLOTS_E  # 768 slots per group
F16 = NSLOT // 16    # 192   (wrapped idx columns)
GF16 = GSLOT // 16   # 48

F32 = mybir.dt.float32
BF16 = mybir.dt.bfloat16
I16 = mybir.dt.int16
U32 = mybir.dt.uint32


def _build():
    nc = bacc.Bacc("TRN2", target_bir_lowering=False, debug=False,
                   num_devices=NCORES)

    x_d = nc.dram_tensor("x", [NT, D], BF16, kind="ExternalInput")
    xTh_d = nc.dram_tensor("xTh", [D, NT], BF16, kind="ExternalInput")
    xTl_d = nc.dram_tensor("xTl", [D, NT], BF16, kind="ExternalInput")
    We_d = nc.dram_tensor("We", [E, D, O], BF16, kind="ExternalInput")
    be_d = nc.dram_tensor("be", [E, O], F32, kind="ExternalInput")
    Wgh_d = nc.dram_tensor("Wgh", [D, E], BF16, kind="ExternalInput")
    Wgl_d = nc.dram_tensor("Wgl", [D, E], BF16, kind="ExternalInput")
    bg_d = nc.dram_tensor("bg", [1, E], F32, kind="ExternalInput")
    idT1_d = nc.dram_tensor("idT1", [E, NT], F32, kind="ExternalInput")
    iota16_d = nc.dram_tensor("iota16", [16, SLOTS_E // 16], F32,
                              kind="ExternalInput")
    out_d = nc.dram_tensor("out", [NT, O], BF16, kind="ExternalOutput")
    sel_scr = nc.dram_tensor("sel_scr", [E * NT], F32, kind="Internal")
    v_scr = nc.dram_tensor("v_scr", [NSLOT], F32, kind="Internal")
    v_scr_w = nc.dram_tensor("v_scr_w", [16, F16], F32, kind="Internal")
    v_rep_scr = nc.dram_tensor("v_rep_scr", [8, 16, F16], F32, kind="Internal")

    with TileContext(nc) as tc:
        with (
            tc.tile_pool(name="const", bufs=1) as const_pool,
            tc.tile_pool(name="wts", bufs=2) as we_pool,
            tc.tile_pool(name="small", bufs=4) as small,
            tc.tile_pool(name="psum_mm", bufs=6, space="PSUM") as psum_mm,
            tc.tile_pool(name="psum_sm", bufs=1, space="PSUM") as psum_sm,
        ):
            # ---- constants / activations ----
            ident = const_pool.tile([P, P], F32, tag="ident")
            make_identity(nc, ident)
            ones_row = const_pool.tile([1, P], F32, tag="ones")
            nc.vector.memset(ones_row, 1.0)

            xTh_sb = const_pool.tile([P, KCH, NT], BF16, tag="xTh")
            xTl_sb = const_pool.tile([P, KCH, NT], BF16, tag="xTl")
            Wgh_sb = const_pool.tile([P, KCH, E], BF16, tag="Wgh")
            nc.scalar.dma_start(out=Wgh_sb,
                                in_=Wgh_d.rearrange("(k p) e -> p k e", p=P))
            Wgl_sb = const_pool.tile([P, KCH, E], BF16, tag="Wgl")
            nc.scalar.dma_start(out=Wgl_sb,
                                in_=Wgl_d.rearrange("(k p) e -> p k e", p=P))
            bg_sb = const_pool.tile([1, E], F32, tag="bg")
            nc.scalar.dma_start(out=bg_sb, in_=bg_d[:, :])
            idT1_sb = const_pool.tile([E, NT], F32, tag="idT1")
            nc.scalar.dma_start(out=idT1_sb, in_=idT1_d[:, :])
            iota16_sb = const_pool.tile([16, SLOTS_E // 16], F32, tag="iota16")
            nc.scalar.dma_start(out=iota16_sb, in_=iota16_d[:, :])
            NQ = NT // 4
            for q in range(4):
                for src_d, dst in ((xTh_d, xTh_sb), (xTl_d, xTl_sb)):
                    nc.sync.dma_start(
                        out=dst[:, :, ds(q * NQ, NQ)],
                        in_=src_d[:, ds(q * NQ, NQ)].rearrange(
                            "(k p) n -> p k n", p=P))
            be_sb = const_pool.tile([E, O], F32, tag="be")
            nc.scalar.dma_start(out=be_sb, in_=be_d[:, :])

            # ---- expert weight streaming ----
            wt_all = {}

            def load_expert(e):
                wt = we_pool.tile([P, KCH, O], BF16, tag="we")
                for h in range(4):
                    nc.sync.dma_start(
                        out=wt[:, ds(h * (KCH // 4), KCH // 4), :],
                        in_=We_d[e, ds(h * (D // 4), D // 4), :].rearrange(
                            "(k p) o -> p k o", p=P))
                wt_all[e] = wt

            load_expert(0)
            load_expert(1)

            # ---- gating: top-2 normalized weights ----
            # w_em[p, e, t]: weight of expert e for token t*128+p (0 if not
            # in top-2).  wT_sb[e, n]: same, expert-major for the bias matmul.
            w_em = const_pool.tile([P, E, TCH], F32, tag="w_em")
            wT_sb = const_pool.tile([E, NT], F32, tag="wT")
            repl = small.tile([P, E], F32, tag="repl", bufs=1)
            nc.vector.memset(repl, -1.0)
            for t in range(TCH):
                psg = psum_sm.tile([P, E], F32, tag="psg", bufs=1)
                for k in range(KCH):
                    nc.tensor.matmul(psg, lhsT=xTh_sb[:, k, ts(t, P)],
                                     rhs=Wgh_sb[:, k, :],
                                     start=(k == 0), stop=False)
                    nc.tensor.matmul(psg, lhsT=xTh_sb[:, k, ts(t, P)],
                                     rhs=Wgl_sb[:, k, :],
                                     start=False, stop=False)
                    nc.tensor.matmul(psg, lhsT=xTl_sb[:, k, ts(t, P)],
                                     rhs=Wgh_sb[:, k, :],
                                     start=False, stop=False)
                nc.tensor.matmul(psg, lhsT=ones_row, rhs=bg_sb,
                                 start=False, stop=True)
                logits = small.tile([P, E], F32, tag="logits")
                nc.scalar.activation(logits, psg,
                                     mybir.ActivationFunctionType.Copy)
                maxes = small.tile([P, E], F32, tag="maxes")
                nc.vector.max(maxes, logits)
                negm1 = small.tile([P, 1], F32, tag="negm1")
                nc.vector.tensor_scalar_mul(negm1, maxes[:, 0:1], -1.0)
                p_ = small.tile([P, E], F32, tag="p")
                nc.scalar.activation(p_, logits,
                                     mybir.ActivationFunctionType.Exp,
                                     bias=negm1, scale=1.0)
                # exp is monotone: top-2 of p = exp(top-2 logits - max),
                # bitwise equal to the p values at those positions
                nc.scalar.activation(repl[:, 0:2], maxes[:, 0:2],
                                     mybir.ActivationFunctionType.Exp,
                                     bias=negm1, scale=1.0)
                denom = small.tile([P, 1], F32, tag="denom")
                nc.vector.tensor_add(denom, repl[:, 0:1], repl[:, 1:2])
                rec = small.tile([P, 1], F32, tag="rec")
                nc.vector.reciprocal(rec, denom)
                pm = small.tile([P, E], F32, tag="pm")
                nc.vector.match_replace(out=pm, in_to_replace=repl,
                                        in_values=p_, imm_value=0.0)
                nc.vector.tensor_sub(pm, p_, pm)  # top-2 values, else 0
                nc.vector.tensor_scalar_mul(w_em[:, :, t], pm, rec)
                pst = psum_sm.tile([E, P], F32, tag="pst", bufs=1)
                nc.tensor.transpose(pst, w_em[:, :, t], ident)
                nc.vector.tensor_copy(wT_sb[:, ts(t, P)], pst)

            # ---- pack (token_id+1 + w/2) where selected, else -1 ----
            # computed on the transposed (expert-major) side [E, NT] so both
            # DRAM bounces below are contiguous; DVE/gpsimd split halves.
            selvT = const_pool.tile([E, NT], F32, tag="selvT")
            maskT = const_pool.tile([E, NT], F32, tag="maskT")
            for h in range(2):
                for sub, eng in ((0, nc.vector), (1, nc.vector)):
                    qs = ds(h * 512 + sub * 256, 256)
                    eng.scalar_tensor_tensor(
                        out=selvT[:, qs], in0=wT_sb[:, qs], scalar=0.25,
                        in1=idT1_sb[:, qs],
                        op0=mybir.AluOpType.mult, op1=mybir.AluOpType.add)
                    eng.tensor_scalar(out=maskT[:, qs], in0=wT_sb[:, qs],
                                      scalar1=0.0, scalar2=None,
                                      op0=mybir.AluOpType.is_gt)
                    eng.tensor_tensor(out=selvT[:, qs], in0=selvT[:, qs],
                                      in1=maskT[:, qs],
                                      op=mybir.AluOpType.mult)
                    eng.tensor_scalar_sub(selvT[:, qs], selvT[:, qs], 1.0)
                # bounce halves out as they finish (linear copy; compaction
                # order inside an expert segment is arbitrary -- packed
                # values self-identify -- so no transpose is needed)
                nc.scalar.dma_start(
                    out=sel_scr.rearrange("(e n) -> e n", e=E)[
                        :, ds(h * 512, 512)],
                    in_=selvT[:, ds(h * 512, 512)])
            sel16 = const_pool.tile([16, E * (NT // 16)], F32, tag="sel16")
            nc.scalar.dma_start(
                out=sel16,
                in_=sel_scr.rearrange("(e p f) -> p e f", p=16, e=E))

            # ---- per-expert compaction (sparse_gather) ----
            nc.gpsimd.load_library(sg_lib)
            v16 = const_pool.tile([16, E, SLOTS_E // 16], F32, tag="v16")
            nf = const_pool.tile([1, E], U32, tag="nf")
            nc.vector.memset(v16, -1.0)
            for e in range(E):
                nc.gpsimd.sparse_gather(
                    v16[:, e, :], sel16[:, ds(e * (NT // 16), NT // 16)],
                    num_found=nf[:, ds(e, 1)])

            # mask pad slots by num_found: ucode sparse_gather leaves
            # garbage (not -1) past the found count.  v' = m*(v+1)-1 with
            # m = (wrapped slot index < n_e).
            nc.gpsimd.load_library(mlp_lib)
            nf16 = const_pool.tile([16, E], U32, tag="nf16")
            nc.gpsimd.partition_broadcast(nf16[:], nf[:])
            nff = const_pool.tile([16, E], F32, tag="nff")
            nc.vector.tensor_copy(nff, nf16)
            for e in range(E):
                m_e = small.tile([16, SLOTS_E // 16], F32, tag="m_e")
                nc.vector.tensor_scalar(out=m_e, in0=iota16_sb,
                                        scalar1=nff[:, ds(e, 1)],
                                        scalar2=None,
                                        op0=mybir.AluOpType.is_lt)
                nc.vector.tensor_scalar_add(v16[:, e, :], v16[:, e, :], 1.0)
                nc.vector.tensor_tensor(out=v16[:, e, :], in0=v16[:, e, :],
                                        in1=m_e, op=mybir.AluOpType.mult)
                nc.vector.tensor_scalar_sub(v16[:, e, :], v16[:, e, :], 1.0)

            v_rep = const_pool.tile([P, F16], F32, tag="v_rep")
            idxs = const_pool.tile([P, F16], I16, tag="idxs")
            xsel = []
            for g in range(NGRP):
                gs = ds(g * GF16, GF16)
                nc.sync.dma_start(out=v_scr_w[:, gs],
                                  in_=v16[:, ds(g * EG, EG), :])
                nc.sync.dma_start(
                    out=v_rep_scr[:, :, gs],
                    in_=v_scr_w[:, gs].unsqueeze(0).broadcast_to(
                        [8, 16, GF16]))
                nc.sync.dma_start(
                    out=v_rep[:, gs],
                    in_=v_rep_scr[:, :, gs].rearrange("a b c -> (a b) c"))
                nc.vector.tensor_scalar_max(v_rep[:, gs], v_rep[:, gs], 0.0)
                nc.vector.tensor_copy(idxs[:, gs], v_rep[:, gs])  # cast
                xs = const_pool.tile([P, KCH, GSLOT], BF16, tag=f"xsel{g}")
                nc.gpsimd.dma_gather(
                    out_ap=xs[:], in_ap=x_d[:],
                    idxs_ap=idxs[:, gs],
                    num_idxs=GSLOT, num_idxs_reg=GSLOT,
                    elem_size=D, transpose=True)
                xsel.append(xs)

            # w per slot-chunk (off the gather critical path)
            nc.scalar.dma_start(
                out=v_scr.rearrange("(s p) -> p s", p=16),
                in_=v16.rearrange("p e f -> p (e f)"))
            v_chunk = const_pool.tile([P, NCHUNK], F32, tag="v_chunk")
            nc.scalar.dma_start(out=v_chunk,
                                in_=v_scr.rearrange("(c p) -> p c", p=P))
            vi = small.tile([P, NCHUNK], I16, tag="vi", bufs=1)
            nc.vector.tensor_copy(vi, v_chunk)
            vf = small.tile([P, NCHUNK], F32, tag="vf", bufs=1)
            nc.vector.tensor_copy(vf, vi)
            w_chunk = const_pool.tile([P, NCHUNK], F32, tag="w_chunk")
            nc.vector.tensor_sub(w_chunk, v_chunk, vf)
            nc.vector.tensor_scalar_mul(w_chunk, w_chunk, 4.0)

            # ---- bias init: out[n] = sum_e w[n,e] * be[e]  (direct write) ----
            y0 = const_pool.tile([P, TCH, O], BF16, tag="y0")
            for t in range(TCH):
                for h in range(OH):
                    psb = psum_mm.tile([P, 512], F32, tag="mm")
                    nc.tensor.matmul(psb, lhsT=wT_sb[:, ts(t, P)],
                                     rhs=be_sb[:, ds(h * 512, 512)],
                                     start=True, stop=True)
                    nc.scalar.activation(y0[:, t, ds(h * 512, 512)], psb,
                                         mybir.ActivationFunctionType.Copy)
            # (write ordered on the Pool queue: after gathers, before
            # the first scatter which accumulates into out)

            for t in range(TCH):
                eng = nc.sync if t % 2 else nc.scalar
                eng.dma_start(
                    out=out_d[ds(t * P, P), :], in_=y0[:, t, :])

            # ---- main: routed expert matmuls + scale ----
            y_grp = None
            for e in range(E):
                if e + 2 < E:
                    load_expert(e + 2)
                wt = wt_all.pop(e)
                g, off0 = e // EG, (e % EG) * SLOTS_E
                if e % EG == 0:
                    y_grp = const_pool.tile([P, EG * CPE, O], BF16,
                                            tag="y", bufs=2)
                for s in range(CPE):
                    c = e * CPE + s
                    lc = (e % EG) * CPE + s
                    for h in range(OH):
                        ps = psum_mm.tile([P, 512], F32, tag="mm")
                        for k in range(KCH):
                            nc.tensor.matmul(
                                ps,
                                lhsT=xsel[g][:, k, ds(off0 + s * P, P)],
                                rhs=wt[:, k, ds(h * 512, 512)],
                                start=(k == 0), stop=(k == KCH - 1))
                        nc.vector.tensor_scalar_mul(
                            y_grp[:, lc, ds(h * 512, 512)], ps,
                            w_chunk[:, ds(c, 1)])
                # one scatter per expert: slots within an expert map to
                # distinct tokens, so no same-row collisions inside one
                # scatter instruction (ucode RMW across DMA engines races);
                # separate instructions are serialized by the out_d dep.
                eh = e % EG
                nc.gpsimd.dma_scatter_add(
                    out_ap=out_d[:],
                    in_ap=y_grp[:, ds(eh * CPE, CPE), :],
                    idxs_ap=idxs[:, ds(g * GF16 + eh * (GF16 // 2),
                                       GF16 // 2)],
                    num_idxs=SLOTS_E, num_idxs_reg=SLOTS_E,
                    elem_size=O)

    nc.compile()
    return nc


_NC_CACHE = None
last_results = None  # BassKernelResults from the most recent run (for test.py)


def _get_nc():
    global _NC_CACHE
    if _NC_CACHE is None:
        _NC_CACHE = _build()
    return _NC_CACHE


def kernel(x, We, be, Wg, bg):
    global last_results
    import ml_dtypes

    bf16 = ml_dtypes.bfloat16

    def hi_lo(a):
        hi = a.astype(bf16)
        lo = (a - hi.astype(np.float32)).astype(bf16)
        return hi, lo

    x = np.asarray(x, dtype=np.float32)
    We_bf = np.ascontiguousarray(np.asarray(We, dtype=np.float32).astype(bf16))
    be_np = np.ascontiguousarray(np.asarray(be, dtype=np.float32))
    Wgh, Wgl = hi_lo(np.asarray(Wg, dtype=np.float32))
    bg_np = np.ascontiguousarray(
        np.asarray(bg, dtype=np.float32)).reshape(1, E)

    # idT1[e, n] = n + 1  (same for every expert row)
    idT1 = np.ascontiguousarray(
        np.tile(np.arange(NT, dtype=np.float32) + 1.0, (E, 1)))
    # wrapped slot index within an expert segment: iota16[p, f] = f*16 + p
    iota16 = np.ascontiguousarray(
        (np.arange(SLOTS_E // 16, dtype=np.float32)[None, :] * 16
         + np.arange(16, dtype=np.float32)[:, None]))

    x_flat = x.reshape(N, D)
    in_maps = []
    for c in range(NCORES):
        xc_f32 = x_flat[c * NT:(c + 1) * NT]
        xc = xc_f32.astype(bf16)
        xTh, xTl = hi_lo(np.ascontiguousarray(xc_f32.T))
        in_maps.append({
            "x": np.ascontiguousarray(xc),
            "xTh": xTh, "xTl": xTl,
            "We": We_bf, "be": be_np, "Wgh": Wgh, "Wgl": Wgl, "bg": bg_np,
            "idT1": idT1, "iota16": iota16,
        })

    last_results = run_bass_kernel_spmd(_get_nc(), in_maps,
                                        core_ids=list(range(NCORES)))
    out = np.concatenate(
        [r["out"].astype(np.float32) for r in last_results.results], axis=0)
    return out.reshape(B, S, O)
